# revision 18
# baseline (speedup 1.0000x reference)
"""Trainium2 Bass/Tile kernel for nn_ActorCriticGAT (2-layer GAT actor-critic).

8 NeuronCores, dst-sharded. See build_plan for the host-side layout scheme:
all sparsity (per-tile interval one-hot stationaries, gather index streams,
per-edge node-feature streams) is host-baked; the device does only dense
matmuls, elementwise pipes, one AllGather (h2 record table), chunked
dma_gather of 256B node records, and one AllReduce for the tiny heads.
"""
import numpy as np

N = 20000
NCORE = 8
DSH = N // NCORE           # 2500
NBLK = (DSH + 127) // 128  # 20
TIL = 128
H, C = 4, 64
GCALL = 1024               # idxs per dma_gather call
CPB = 4                    # gather calls per processing batch (32 tiles)

_graph_cache = {}


def _bf16(a):
    import ml_dtypes
    return np.ascontiguousarray(np.asarray(a, dtype=np.float32)).astype(ml_dtypes.bfloat16)


def _wrap16(idx):
    n = len(idx)
    blk = idx.reshape(n // 16, 16).T.astype(np.int16)
    return np.tile(blk, (8, 1))


def build_plan(inputs):
    x = np.asarray(inputs["node_features"], np.float32)
    ei = np.asarray(inputs["edge_index"])
    src_all = ei[0].astype(np.int64)
    dst_all = ei[1].astype(np.int64)
    cni = int(np.asarray(inputs["current_node_idx"]))
    W1 = np.asarray(inputs["W1"], np.float32)
    W2 = np.asarray(inputs["W2"], np.float32)

    cores = []
    TB = 0
    for k in range(NCORE):
        lo = k * DSH
        m = (dst_all >= lo) & (dst_all < lo + DSH)
        es, ed = src_all[m], dst_all[m]
        o = np.argsort(ed, kind="stable")
        es, ed = es[o], ed[o] - lo
        cnt = np.bincount(ed // 128, minlength=NBLK)
        TB = max(TB, int(np.ceil(cnt / TIL).max()))
        cores.append((es, ed, cnt))
    NT = NBLK * TB
    NT = ((NT + (CPB * 8) - 1) // (CPB * 8)) * (CPB * 8)  # align to batch (32 tiles)
    NCALL = NT * TIL // GCALL

    per_core = []
    for k in range(NCORE):
        es, ed, cnt = cores[k]
        Ep = NT * TIL
        src_p = np.zeros(Ep, dtype=np.int64)
        dst_p = np.zeros(Ep, dtype=np.int64)
        valid = np.zeros(Ep, dtype=bool)
        tile_blk = np.full(NT, NBLK - 1, dtype=np.int64)
        ofs = np.concatenate([[0], np.cumsum(cnt)])
        for b in range(NBLK):
            t0 = b * TB
            tile_blk[t0:t0 + TB] = b
            nb = int(cnt[b])
            sl = slice(t0 * TIL, t0 * TIL + nb)
            src_p[sl] = es[ofs[b]:ofs[b + 1]]
            dst_p[sl] = ed[ofs[b]:ofs[b + 1]]
            valid[sl] = True

        e_idx = np.arange(NT * TIL)
        doff = dst_p - tile_blk[e_idx // TIL] * 128
        AGG = np.zeros((NT, TIL, TIL), dtype=np.float32)
        AGG[e_idx // TIL, e_idx % TIL, np.where(valid, doff, 0)] = valid.astype(np.float32)
        EXPT = AGG.transpose(0, 2, 1).copy()

        xs = np.where(valid[:, None], x[src_p], 0.0)
        xd = np.where(valid[:, None], x[np.minimum(dst_p + k * DSH, N - 1)], 0.0)
        XSE = np.concatenate([xs, np.ones((Ep, 1), np.float32)], 1)
        XSE = XSE.reshape(NT, TIL, 4).transpose(1, 0, 2)
        XDE = xd.reshape(NT, TIL, 3).transpose(1, 0, 2)

        widx = np.zeros((128, NT * 8), dtype=np.int16)
        for g in range(NCALL):
            widx[:, g * 64:(g + 1) * 64] = _wrap16(src_p[g * GCALL:(g + 1) * GCALL])

        selones = np.zeros((NBLK * 128, 2), np.float32)
        selones[:DSH, 1] = 1.0
        if k * DSH <= cni < (k + 1) * DSH:
            selones[cni - k * DSH, 0] = 1.0
        selones = selones.reshape(NBLK, 128, 2).transpose(1, 0, 2)

        per_core.append(dict(
            AGG=_bf16(AGG.transpose(1, 0, 2)),
            EXPT=_bf16(EXPT.transpose(1, 0, 2)),
            XSE=_bf16(XSE), XDE=_bf16(XDE),
            IDX=widx, SEL=_bf16(np.ascontiguousarray(selones)),
        ))

    W1r = W1.reshape(3, H, C)
    V = np.zeros((12, 256), np.float32)
    for h in range(H):
        V[h * 3:(h + 1) * 3, h * C:(h + 1) * C] = W1r[:, h, :]
    shared = dict(
        V=_bf16(V),
        W2CB=_bf16(W2.reshape(2, 128, C).transpose(1, 0, 2)),
        A2D2=_bf16(np.stack([np.asarray(inputs["a_src2"], np.float32)[0],
                             np.asarray(inputs["a_dst2"], np.float32)[0]], 1)),
        B1SB=np.ascontiguousarray(np.asarray(inputs["b1"], np.float32).reshape(2, 128).T),
        B2ROW=np.asarray(inputs["b2"], np.float32).reshape(1, C),
        WPT=np.ascontiguousarray(np.asarray(inputs["Wp"], np.float32).T),
        WVT=np.ascontiguousarray(np.asarray(inputs["Wv"], np.float32).T),
        BP=np.asarray(inputs["bp"], np.float32).reshape(1, 2),
        BV=np.asarray(inputs["bv"], np.float32).reshape(1, 1),
        W1SB=np.ascontiguousarray(W1.reshape(3, 256)),
        A1F=np.asarray(inputs["a_src1"], np.float32).reshape(1, 256),
        D1F=np.asarray(inputs["a_dst1"], np.float32).reshape(1, 256),
        IDEN=_bf16(np.eye(128, dtype=np.float32)),
        IDENF=np.eye(4, dtype=np.float32),
    )
    in_maps = []
    for k in range(NCORE):
        d = dict(shared)
        d.update(per_core[k])
        in_maps.append(d)
    return NT, NCALL, TB, in_maps


def build_graph(NT, NCALL, TB):
    import concourse.bass as bass
    import concourse.bacc as bacc
    import concourse.tile as tile
    import concourse.mybir as mybir
    import bass_rust
    from concourse._compat import get_trn_type
    from contextlib import ExitStack

    dt = mybir.dt
    AT = mybir.AluOpType
    AF = mybir.ActivationFunctionType
    AX = bass_rust.AxisListType

    nc = bacc.Bacc(get_trn_type() or "TRN2", num_swdge_queues=4)
    P = {}
    def par(name, shape, dtype):
        P[name] = nc.dram_tensor(name, shape, dtype, kind="ExternalInput")
    par("AGG", [128, NT, 128], dt.bfloat16)
    par("EXPT", [128, NT, 128], dt.bfloat16)
    par("XSE", [128, NT, 4], dt.bfloat16)
    par("XDE", [128, NT, 3], dt.bfloat16)
    par("IDX", [128, NT * 8], dt.int16)
    par("SEL", [128, NBLK, 2], dt.bfloat16)
    par("V", [12, 256], dt.bfloat16)
    par("W2CB", [128, 2, C], dt.bfloat16)
    par("A2D2", [C, 2], dt.bfloat16)
    par("B1SB", [128, 2], dt.float32)
    par("B2ROW", [1, C], dt.float32)
    par("WPT", [2, C], dt.float32)
    par("WVT", [1, C], dt.float32)
    par("BP", [1, 2], dt.float32)
    par("BV", [1, 1], dt.float32)
    par("W1SB", [3, 256], dt.float32)
    par("A1F", [1, 256], dt.float32)
    par("D1F", [1, 256], dt.float32)
    par("IDEN", [128, 128], dt.bfloat16)
    par("IDENF", [4, 4], dt.float32)
    out_ext = nc.dram_tensor("out", [1, 3], dt.float32, kind="ExternalOutput")
    dbg1_ext = nc.dram_tensor("dbg1", [128, NBLK, 16], dt.float32, kind="ExternalOutput")
    dbg2_ext = nc.dram_tensor("dbg2", [128, NBLK, 66], dt.float32, kind="ExternalOutput")
    dbg3_ext = nc.dram_tensor("dbg3", [128, 64], dt.float32, kind="ExternalOutput")
    dbg4_ext = nc.dram_tensor("dbg4", [128, 128], dt.bfloat16, kind="ExternalOutput")
    dbg5_ext = nc.dram_tensor("dbg5", [128, 128], dt.bfloat16, kind="ExternalOutput")

    recbounce = nc.dram_tensor("recbounce", [DSH, 128], dt.bfloat16)
    table = nc.dram_tensor("tableag", [N, 128], dt.bfloat16, addr_space="Shared")
    finloc = nc.dram_tensor("finloc", [2, C], dt.float32)
    finsh = nc.dram_tensor("finsh", [2, C], dt.float32, addr_space="Shared")

    NCH = (NBLK * 128) // 512   # 5
    NBATCH = NCALL // CPB

    with nc.semaphore("cs") as cs, nc.semaphore("cc") as cc, \
         tile.TileContext(nc) as tc, ExitStack() as ctx:
        pool = ctx.enter_context(tc.tile_pool(name="persist", bufs=1))
        stream = ctx.enter_context(tc.tile_pool(name="stream", bufs=2))
        ring = ctx.enter_context(tc.tile_pool(name="ring", bufs=2))
        s2p = ctx.enter_context(tc.tile_pool(name="s2p", bufs=2))
        pps = ctx.enter_context(tc.tile_pool(name="pps", bufs=2, space="PSUM"))
        ppb = pps
        ppq = ctx.enter_context(tc.tile_pool(name="ppq", bufs=2, space="PSUM"))

        def T(name, shape, dtype):
            t = pool.tile(shape, dtype, tag=name)
            return t

        # persistent SBUF tiles + input loads
        loads = [
            ("XSE", [128, NT, 4], dt.bfloat16), ("XDE", [128, NT, 3], dt.bfloat16),
            ("IDX", [128, NT * 8], dt.int16), ("SEL", [128, NBLK, 2], dt.bfloat16),
            ("V", [12, 256], dt.bfloat16), ("W2CB", [128, 2, C], dt.bfloat16),
            ("A2D2", [C, 2], dt.bfloat16), ("B1SB", [128, 2], dt.float32),
            ("B2ROW", [1, C], dt.float32), ("WPT", [2, C], dt.float32),
            ("WVT", [1, C], dt.float32), ("BP", [1, 2], dt.float32),
            ("BV", [1, 1], dt.float32), ("W1SB", [3, 256], dt.float32),
            ("A1F", [1, 256], dt.float32), ("D1F", [1, 256], dt.float32),
            ("IDEN", [128, 128], dt.bfloat16), ("IDENF", [4, 4], dt.float32),
        ]
        S = {}
        for name, shape, dtype in loads:
            S[name] = T(name, shape, dtype)
            nc.sync.dma_start(out=S[name][:], in_=P[name][:])

        TT = nc.vector.tensor_tensor
        ACT = nc.scalar.activation

        # ---- v1/u1 = einsum(W1,[a_src1|a_dst1]) -> broadcast to all partitions
        tmp = pool.tile([3, 256], dt.float32, tag="tmp")
        v1s = pool.tile([3, 4], dt.float32, tag="v1s")
        u1s = pool.tile([3, 4], dt.float32, tag="u1s")
        vpack = pool.tile([1, 24], dt.float32, tag="vpack")
        ones1 = pool.tile([1, 128], dt.float32, tag="ones1")
        v1bc = pool.tile([128, 24], dt.float32, tag="v1bc")
        nc.vector.memset(ones1[:], 1.0)
        rep = pool.tile([3, 256], dt.float32, tag="rep")
        for f in range(3):
            nc.gpsimd.dma_start(out=rep[f:f + 1, :], in_=S["A1F"][:])
        TT(out=tmp[:], in0=S["W1SB"][:], in1=rep[:], op=AT.mult)
        nc.vector.reduce_sum(out=v1s[:].rearrange("f (h o) -> f h o", o=1),
                             in_=tmp[:].rearrange("f (h c) -> f h c", h=4), axis=AX.X)
        for f in range(3):
            nc.gpsimd.dma_start(out=rep[f:f + 1, :], in_=S["D1F"][:])
        TT(out=tmp[:], in0=S["W1SB"][:], in1=rep[:], op=AT.mult)
        nc.vector.reduce_sum(out=u1s[:].rearrange("f (h o) -> f h o", o=1),
                             in_=tmp[:].rearrange("f (h c) -> f h c", h=4), axis=AX.X)
        nc.gpsimd.dma_start(out=vpack[0:1, 0:12], in_=v1s[:])
        nc.gpsimd.dma_start(out=vpack[0:1, 12:24], in_=u1s[:])
        pv = pps.tile([128, 24], dt.float32, tag="ps")
        nc.tensor.matmul(out=pv[:], lhsT=ones1[:], rhs=vpack[:], start=True, stop=True)
        ACT(out=v1bc[:], in_=pv[:], func=AF.Copy)

        # ---- layer-1 per-edge attention scalars
        AS1 = T("AS1", [128, NT, 4], dt.bfloat16)
        AD1 = T("AD1", [128, NT, 4], dt.bfloat16)
        TMP4 = T("TMP4", [128, NT, 4], dt.bfloat16)
        S1 = T("S1", [128, NT, 4], dt.bfloat16)
        MSG1 = T("MSG1", [128, NT, 16], dt.bfloat16)
        for (dstt, srcs, base) in [(AS1, S["XSE"], 0), (AD1, S["XDE"], 12)]:
            for f in range(3):
                t_in0 = srcs[:, :, f:f + 1].broadcast_to([128, NT, 4])
                t_sc = v1bc[:, base + f * 4:base + f * 4 + 4] \
                    .rearrange("p (o h) -> p o h", o=1).broadcast_to([128, NT, 4])
                if f == 0:
                    TT(out=dstt[:], in0=t_in0, in1=t_sc, op=AT.mult)
                else:
                    TT(out=TMP4[:], in0=t_in0, in1=t_sc, op=AT.mult)
                    TT(out=dstt[:], in0=dstt[:], in1=TMP4[:], op=AT.add)
        TT(out=AS1[:], in0=AS1[:], in1=AD1[:], op=AT.add)      # e1
        nc.vector.tensor_scalar_mul(out=AD1[:], in0=AS1[:], scalar1=0.2)
        TT(out=AS1[:], in0=AS1[:], in1=AD1[:], op=AT.max)      # lrelu(e1)
        ACT(out=S1[:], in_=AS1[:], func=AF.Exp)
        TT(out=MSG1[:],
           in0=S1[:].rearrange("p t (h o) -> p t h o", o=1).broadcast_to([128, NT, 4, 4]),
           in1=S["XSE"][:].rearrange("p (t o) f -> p t o f", o=1).broadcast_to([128, NT, 4, 4]),
           op=AT.mult)

        # ---- layer-1 aggregation (AGG-streamed interval matmuls)
        Q1 = T("Q1", [128, NBLK, 16], dt.float32)
        blk_of = lambda t: min(t // TB, NBLK - 1)
        pq = None
        for bb in range(NT // 32):
            ag = stream.tile([128, 32, 128], dt.bfloat16, tag="aggs")
            nc.sync.dma_start(out=ag[:], in_=P["AGG"][:, bb * 32:(bb + 1) * 32, :])
            for j in range(32):
                t = bb * 32 + j
                b = blk_of(t)
                first = (t == b * TB) or (b == NBLK - 1 and t == (NBLK - 1) * TB)
                last = (t == (b + 1) * TB - 1) or (t == NT - 1)
                if b == NBLK - 1:
                    last = (t == NT - 1)
                if first:
                    pq = ppq.tile([128, 16], dt.float32, tag="pq")
                nc.tensor.matmul(out=pq[:], lhsT=ag[:, j, :], rhs=MSG1[:, t, :],
                                 start=first, stop=last)
                if last:
                    ACT(out=Q1[:, b, :], in_=pq[:], func=AF.Copy)

        # ---- normalize, out1 = relu(yn @ V + b1), h2 = x1 @ W2, as2/ad2
        ZR = T("ZR", [128, NBLK, 4], dt.float32)
        YN = T("YN", [128, NBLK, 16], dt.bfloat16)
        nc.vector.tensor_scalar_add(out=ZR[:], in0=Q1[:, :, 3::4], scalar1=1e-16)
        nc.vector.reciprocal(out=ZR[:], in_=ZR[:])
        TT(out=YN[:], in0=Q1[:],
           in1=ZR[:].rearrange("p b (h o) -> p b h o", o=1).broadcast_to([128, NBLK, 4, 4]),
           op=AT.mult)
        YNT = T("YNT", [12, NBLK * 128], dt.bfloat16)
        YN12 = T("YN12", [128, NBLK, 12], dt.bfloat16)
        nc.vector.tensor_copy(out=YN12[:].rearrange("p b (h f) -> p b h f", h=4),
                              in_=YN[:].rearrange("p b (h f) -> p b h f", h=4)[:, :, :, 0:3])
        for b in range(NBLK):
            pt = pps.tile([12, 128], dt.bfloat16, tag="ps")
            nc.tensor.transpose(out=pt[:], in_=YN12[:, b, :], identity=S["IDEN"][:])
            ACT(out=YNT[:, b * 128:(b + 1) * 128], in_=pt[:], func=AF.Copy)
        X1T = T("X1T", [128, 2, NBLK * 128], dt.bfloat16)
        for j in range(2):
            for chv in range(NCH):
                pb = ppb.tile([128, 512], dt.float32, tag="ps")
                nc.tensor.matmul(out=pb[:], lhsT=S["V"][:, j * 128:(j + 1) * 128],
                                 rhs=YNT[:, chv * 512:(chv + 1) * 512], start=True, stop=True)
                ACT(out=X1T[:, j, chv * 512:(chv + 1) * 512], in_=pb[:], func=AF.Relu,
                    bias=S["B1SB"][:, j:j + 1], scale=1.0)
        H2T = T("H2T", [C, NBLK * 128], dt.bfloat16)
        for chv in range(NCH):
            pb = ppb.tile([64, 512], dt.float32, tag="ps")
            for j in range(2):
                nc.tensor.matmul(out=pb[:], lhsT=S["W2CB"][:, j, :],
                                 rhs=X1T[:, j, chv * 512:(chv + 1) * 512],
                                 start=(j == 0), stop=(j == 1))
            ACT(out=H2T[:, chv * 512:(chv + 1) * 512], in_=pb[:], func=AF.Copy)
        ASD = T("ASD", [2, NBLK * 128], dt.float32)
        for chv in range(NCH):
            pb = pps.tile([2, 512], dt.float32, tag="ps")
            nc.tensor.matmul(out=pb[:], lhsT=S["A2D2"][:], rhs=H2T[:, chv * 512:(chv + 1) * 512],
                             start=True, stop=True)
            ACT(out=ASD[:, chv * 512:(chv + 1) * 512], in_=pb[:], func=AF.Copy)

        # ---- records [h2 | 1 | as2 | 0pad] bf16 and table allgather
        REC = T("REC", [128, NBLK, 128], dt.bfloat16)
        nc.vector.memset(REC[:, :, 64:128], 0.0)
        nc.vector.memset(REC[:, :, 64:65], 1.0)
        for b in range(NBLK):
            pt = pps.tile([128, 64], dt.bfloat16, tag="ps")
            nc.tensor.transpose(out=pt[:], in_=H2T[:, b * 128:(b + 1) * 128], identity=S["IDEN"][0:64, 0:64])
            ACT(out=REC[:, b, 0:64], in_=pt[:], func=AF.Copy)
        AS2C = T("AS2C", [128, NBLK], dt.float32)
        AD2C = T("AD2C", [128, NBLK], dt.bfloat16)
        ASDC = T("ASDC", [128, NBLK, 2], dt.float32)
        for b in range(NBLK):
            pt2 = pps.tile([128, 2], dt.float32, tag="ps")
            nc.tensor.transpose(out=pt2[:], in_=ASD[:, b * 128:(b + 1) * 128], identity=S["IDENF"][0:2, 0:2])
            ACT(out=ASDC[:, b, :], in_=pt2[:], func=AF.Copy)
        nc.vector.tensor_copy(out=AS2C[:], in_=ASDC[:, :, 0])
        nc.vector.tensor_copy(out=AD2C[:], in_=ASDC[:, :, 1])
        nc.vector.tensor_copy(out=REC[:, :, 65:66], in_=AS2C[:].rearrange("p (b o) -> p b o", o=1))

        with tc.tile_critical():
            nc.gpsimd.dma_start(out=recbounce[0:19 * 128, :].rearrange("(b p) c -> p b c", p=128),
                                in_=REC[:, 0:19, :]).then_inc(cs, 16)
            nc.gpsimd.dma_start(out=recbounce[19 * 128:DSH, :],
                                in_=REC[0:DSH - 19 * 128, 19, :]).then_inc(cs, 16)
            nc.gpsimd.wait_ge(cs, 32)
            nc.gpsimd.collective_compute(
                "AllGather", AT.bypass, replica_groups=[list(range(NCORE))],
                ins=[recbounce[:]], outs=[table[:]]).then_inc(cc, 1)
            nc.gpsimd.wait_ge(cc, 1)

        # ---- ad2 per-edge expansion via transposed interval matmuls
        AD2E = T("AD2E", [128, NT], dt.float32)
        for bb in range(NT // 32):
            ex = stream.tile([128, 32, 128], dt.bfloat16, tag="aggs")
            nc.sync.dma_start(out=ex[:], in_=P["EXPT"][:, bb * 32:(bb + 1) * 32, :])
            pb = ppb.tile([128, 32], dt.float32, tag="ps")
            for j in range(32):
                t = bb * 32 + j
                b = blk_of(t)
                nc.tensor.matmul(out=pb[:, j:j + 1], lhsT=ex[:, j, :],
                                 rhs=AD2C[:, b:b + 1], start=True, stop=True)
            ACT(out=AD2E[:, bb * 32:(bb + 1) * 32], in_=pb[:], func=AF.Copy)

        # ---- layer-2: gather + softmax scale + aggregation
        Q2 = T("Q2", [128, NBLK, 66], dt.float32)
        pq2 = None
        for bt in range(NBATCH):
            gr = ring.tile([128, CPB * 8, 128], dt.bfloat16, tag="gr")
            for cll in range(CPB):
                g = bt * CPB + cll
                nc.gpsimd.dma_gather(
                    gr[:, cll * 8:(cll + 1) * 8, :], table[:, :],
                    S["IDX"][:, g * 64:(g + 1) * 64],
                    GCALL, GCALL, 128, queue_num=g % 4)
            s2 = s2p.tile([128, CPB * 8], dt.float32, tag="s2")
            s2b = s2p.tile([128, CPB * 8], dt.float32, tag="s2")
            a2c = gr[:, :, 65:66].rearrange("p t o -> p (t o)")
            e2c = AD2E[:, bt * 32:(bt + 1) * 32]
            TT(out=s2[:], in0=a2c, in1=e2c, op=AT.add)
            nc.vector.tensor_scalar_mul(out=s2b[:], in0=s2[:], scalar1=0.2)
            TT(out=s2[:], in0=s2[:], in1=s2b[:], op=AT.max)
            ACT(out=s2[:], in_=s2[:], func=AF.Exp)
            TT(out=gr[:, :, 0:66], in0=gr[:, :, 0:66],
               in1=s2[:].rearrange("p (t o) -> p t o", o=1).broadcast_to([128, CPB * 8, 66]),
               op=AT.mult)
            ag = stream.tile([128, 32, 128], dt.bfloat16, tag="aggs")
            nc.sync.dma_start(out=ag[:], in_=P["AGG"][:, bt * 32:(bt + 1) * 32, :])
            for j in range(32):
                t = bt * 32 + j
                b = blk_of(t)
                first = (t == b * TB) or (b == NBLK - 1 and t == (NBLK - 1) * TB)
                last = (t == (b + 1) * TB - 1) or (t == NT - 1)
                if b == NBLK - 1:
                    last = (t == NT - 1)
                if first:
                    pq2 = ppq.tile([128, 66], dt.float32, tag="pq")
                nc.tensor.matmul(out=pq2[:], lhsT=ag[:, j, :], rhs=gr[:, j, 0:66],
                                 start=first, stop=last)
                if last:
                    ACT(out=Q2[:, b, :], in_=pq2[:], func=AF.Copy)

        # ---- emb, heads partials, allreduce, final
        B2BC = T("B2BC", [128, C], dt.float32)
        pb2 = pps.tile([128, C], dt.float32, tag="ps")
        nc.tensor.matmul(out=pb2[:], lhsT=ones1[:], rhs=S["B2ROW"][:], start=True, stop=True)
        ACT(out=B2BC[:], in_=pb2[:], func=AF.Copy)
        EMB = T("EMB", [128, NBLK, C], dt.bfloat16)
        ZR2 = T("ZR2", [128, NBLK, 1], dt.float32)
        nc.vector.tensor_scalar_add(out=ZR2[:], in0=Q2[:, :, 64:65], scalar1=1e-16)
        nc.vector.reciprocal(out=ZR2[:], in_=ZR2[:])
        TT(out=EMB[:], in0=Q2[:, :, 0:64], in1=ZR2[:].broadcast_to([128, NBLK, 64]), op=AT.mult)
        TT(out=EMB[:], in0=EMB[:],
           in1=B2BC[:].rearrange("p (z c) -> p z c", z=1).broadcast_to([128, NBLK, 64]),
           op=AT.add)
        pf = pps.tile([2, C], dt.float32, tag="ps")
        for b in range(NBLK):
            nc.tensor.matmul(out=pf[:], lhsT=S["SEL"][:, b, :], rhs=EMB[:, b, :],
                             start=(b == 0), stop=(b == NBLK - 1))
        FIN = T("FIN", [2, C], dt.float32)
        SC1 = T("SC1", [2, C], dt.float32)
        ACT(out=FIN[:], in_=pf[:], func=AF.Copy)
        with tc.tile_critical():
            nc.gpsimd.dma_start(out=finloc[:], in_=FIN[:]).then_inc(cs, 16)
            nc.gpsimd.wait_ge(cs, 48)
            nc.gpsimd.collective_compute(
                "AllReduce", AT.add, replica_groups=[list(range(NCORE))],
                ins=[finloc[:]], outs=[finsh[:]]).then_inc(cc, 1)
            nc.gpsimd.wait_ge(cc, 2)
            nc.gpsimd.dma_start(out=SC1[:], in_=finsh[:]).then_inc(cs, 16)
            nc.gpsimd.wait_ge(cs, 64)

        WPR = T("WPR", [1, 2 * C], dt.float32)
        nc.gpsimd.dma_start(out=WPR[:], in_=S["WPT"][:])
        SC1F = T("SC1F", [1, 2 * C], dt.float32)
        nc.gpsimd.dma_start(out=SC1F[:], in_=SC1[:])
        TMP2 = T("TMP2", [2, C], dt.float32)
        SC2 = T("SC2", [2, 1], dt.float32)
        SC3 = T("SC3", [1, 4], dt.float32)
        OUTS = T("OUTS", [1, 3], dt.float32)
        for j in range(2):
            TT(out=TMP2[0:1, :], in0=WPR[0:1, j * C:(j + 1) * C], in1=SC1F[0:1, 0:C], op=AT.mult)
            nc.vector.reduce_sum(out=SC3[0:1, j:j + 1], in_=TMP2[0:1, :], axis=AX.X)
        TT(out=SC3[0:1, 0:2], in0=SC3[0:1, 0:2], in1=S["BP"][:], op=AT.add)
        TT(out=TMP2[0:1, :], in0=S["WVT"][:], in1=SC1F[0:1, C:2 * C], op=AT.mult)
        nc.vector.reduce_sum(out=SC3[0:1, 2:3], in_=TMP2[0:1, :], axis=AX.X)
        nc.vector.tensor_scalar_mul(out=SC3[0:1, 2:3], in0=SC3[0:1, 2:3], scalar1=1.0 / N)
        TT(out=SC3[0:1, 2:3], in0=SC3[0:1, 2:3], in1=S["BV"][:], op=AT.add)
        ACT(out=SC3[0:1, 0:2], in_=SC3[0:1, 0:2], func=AF.Exp)
        nc.vector.reduce_sum(out=SC3[0:1, 3:4], in_=SC3[0:1, 0:2], axis=AX.X)
        nc.vector.reciprocal(out=SC3[0:1, 3:4], in_=SC3[0:1, 3:4])
        TT(out=OUTS[0:1, 0:2], in0=SC3[0:1, 0:2],
           in1=SC3[0:1, 3:4].broadcast_to([1, 2]), op=AT.mult)
        nc.vector.tensor_copy(out=OUTS[0:1, 2:3], in_=SC3[0:1, 2:3])
        nc.gpsimd.dma_start(out=out_ext[:], in_=OUTS[:])
        nc.gpsimd.dma_start(out=dbg1_ext[:], in_=Q1[:])
        nc.gpsimd.dma_start(out=dbg2_ext[:], in_=Q2[:])
        nc.gpsimd.dma_start(out=dbg3_ext[:, 0:NBLK], in_=AD2E[:, 0:NBLK])
        nc.gpsimd.dma_start(out=dbg4_ext[:], in_=REC[:, 0, :])
        nc.gpsimd.dma_start(out=dbg5_ext[:], in_=table[128:256, :])

    nc.compile()
    return nc


def kernel(**inputs):
    from concourse.bass_utils import run_bass_kernel_spmd
    NT, NCALL, TB, in_maps = build_plan(inputs)
    key = (NT, NCALL, TB)
    if key not in _graph_cache:
        _graph_cache[key] = build_graph(NT, NCALL, TB)
    nc = _graph_cache[key]
    res = run_bass_kernel_spmd(nc, in_maps, list(range(NCORE)))
    o = np.asarray(res.results[0]["out"], dtype=np.float32).reshape(3)
    return np.asarray(o[0:2], dtype=np.float32), np.asarray(o[2:3], dtype=np.float32)


# revision 19
# speedup vs baseline: 1.2891x; 1.2891x over previous
"""Trainium2 Bass/Tile kernel for nn_ActorCriticGAT (2-layer GAT actor-critic).

8 NeuronCores, dst-sharded. See build_plan for the host-side layout scheme:
all sparsity (per-tile interval one-hot stationaries, gather index streams,
per-edge node-feature streams) is host-baked; the device does only dense
matmuls, elementwise pipes, one AllGather (h2 record table), chunked
dma_gather of 256B node records, and one AllReduce for the tiny heads.
"""
import numpy as np

N = 20000
NCORE = 8
DSH = N // NCORE           # 2500
NBLK = (DSH + 127) // 128  # 20
TIL = 128
H, C = 4, 64
GCALL = 1024               # idxs per dma_gather call
CPB = 4                    # gather calls per processing batch (32 tiles)

_graph_cache = {}


def _bf16(a):
    import ml_dtypes
    return np.ascontiguousarray(np.asarray(a, dtype=np.float32)).astype(ml_dtypes.bfloat16)


def _wrap16(idx):
    n = len(idx)
    blk = idx.reshape(n // 16, 16).T.astype(np.int16)
    return np.tile(blk, (8, 1))


def build_plan(inputs):
    x = np.asarray(inputs["node_features"], np.float32)
    ei = np.asarray(inputs["edge_index"])
    src_all = ei[0].astype(np.int64)
    dst_all = ei[1].astype(np.int64)
    cni = int(np.asarray(inputs["current_node_idx"]))
    W1 = np.asarray(inputs["W1"], np.float32)
    W2 = np.asarray(inputs["W2"], np.float32)

    cores = []
    TB = 0
    for k in range(NCORE):
        lo = k * DSH
        m = (dst_all >= lo) & (dst_all < lo + DSH)
        es, ed = src_all[m], dst_all[m]
        o = np.argsort(ed, kind="stable")
        es, ed = es[o], ed[o] - lo
        cnt = np.bincount(ed // 128, minlength=NBLK)
        TB = max(TB, int(np.ceil(cnt / TIL).max()))
        cores.append((es, ed, cnt))
    NT = NBLK * TB
    NT = ((NT + (CPB * 8) - 1) // (CPB * 8)) * (CPB * 8)  # align to batch (32 tiles)
    NCALL = NT * TIL // GCALL

    per_core = []
    for k in range(NCORE):
        es, ed, cnt = cores[k]
        Ep = NT * TIL
        src_p = np.zeros(Ep, dtype=np.int64)
        dst_p = np.zeros(Ep, dtype=np.int64)
        valid = np.zeros(Ep, dtype=bool)
        tile_blk = np.full(NT, NBLK - 1, dtype=np.int64)
        ofs = np.concatenate([[0], np.cumsum(cnt)])
        for b in range(NBLK):
            t0 = b * TB
            tile_blk[t0:t0 + TB] = b
            nb = int(cnt[b])
            sl = slice(t0 * TIL, t0 * TIL + nb)
            src_p[sl] = es[ofs[b]:ofs[b + 1]]
            dst_p[sl] = ed[ofs[b]:ofs[b + 1]]
            valid[sl] = True

        e_idx = np.arange(NT * TIL)
        doff = dst_p - tile_blk[e_idx // TIL] * 128
        AGG = np.zeros((NT, TIL, TIL), dtype=np.float32)
        AGG[e_idx // TIL, e_idx % TIL, np.where(valid, doff, 0)] = valid.astype(np.float32)
        EXPT = AGG.transpose(0, 2, 1).copy()

        xs = np.where(valid[:, None], x[src_p], 0.0)
        xd = np.where(valid[:, None], x[np.minimum(dst_p + k * DSH, N - 1)], 0.0)
        XSE = np.concatenate([xs, np.ones((Ep, 1), np.float32)], 1)
        XSE = XSE.reshape(NT, TIL, 4).transpose(1, 0, 2)
        XDE = xd.reshape(NT, TIL, 3).transpose(1, 0, 2)

        widx = np.zeros((128, NT * 8), dtype=np.int16)
        for g in range(NCALL):
            widx[:, g * 64:(g + 1) * 64] = _wrap16(src_p[g * GCALL:(g + 1) * GCALL])

        selones = np.zeros((NBLK * 128, 2), np.float32)
        selones[:DSH, 1] = 1.0
        if k * DSH <= cni < (k + 1) * DSH:
            selones[cni - k * DSH, 0] = 1.0
        selones = selones.reshape(NBLK, 128, 2).transpose(1, 0, 2)

        import ml_dtypes
        per_core.append(dict(
            AGG=AGG.transpose(1, 0, 2).astype(ml_dtypes.float8_e4m3),
            EXPT=EXPT.transpose(1, 0, 2).astype(ml_dtypes.float8_e4m3),
            XSE=_bf16(XSE), XDE=_bf16(XDE),
            IDX=widx, SEL=_bf16(np.ascontiguousarray(selones)),
        ))

    W1r = W1.reshape(3, H, C)
    V = np.zeros((12, 256), np.float32)
    for h in range(H):
        V[h * 3:(h + 1) * 3, h * C:(h + 1) * C] = W1r[:, h, :]
    shared = dict(
        V=_bf16(V),
        W2CB=_bf16(W2.reshape(2, 128, C).transpose(1, 0, 2)),
        A2D2=_bf16(np.stack([np.asarray(inputs["a_src2"], np.float32)[0],
                             np.asarray(inputs["a_dst2"], np.float32)[0]], 1)),
        B1SB=np.ascontiguousarray(np.asarray(inputs["b1"], np.float32).reshape(2, 128).T),
        B2ROW=np.asarray(inputs["b2"], np.float32).reshape(1, C),
        WPT=np.ascontiguousarray(np.asarray(inputs["Wp"], np.float32).T),
        WVT=np.ascontiguousarray(np.asarray(inputs["Wv"], np.float32).T),
        BP=np.asarray(inputs["bp"], np.float32).reshape(1, 2),
        BV=np.asarray(inputs["bv"], np.float32).reshape(1, 1),
        W1SB=np.ascontiguousarray(W1.reshape(3, 256)),
        A1F=np.asarray(inputs["a_src1"], np.float32).reshape(1, 256),
        D1F=np.asarray(inputs["a_dst1"], np.float32).reshape(1, 256),
        IDEN=_bf16(np.eye(128, dtype=np.float32)),
        IDENF=np.eye(4, dtype=np.float32),
    )
    in_maps = []
    for k in range(NCORE):
        d = dict(shared)
        d.update(per_core[k])
        in_maps.append(d)
    return NT, NCALL, TB, in_maps


def build_graph(NT, NCALL, TB):
    import concourse.bass as bass
    import concourse.bacc as bacc
    import concourse.tile as tile
    import concourse.mybir as mybir
    import bass_rust
    from concourse._compat import get_trn_type
    from contextlib import ExitStack

    dt = mybir.dt
    AT = mybir.AluOpType
    AF = mybir.ActivationFunctionType
    AX = bass_rust.AxisListType

    nc = bacc.Bacc(get_trn_type() or "TRN2", num_swdge_queues=4)
    P = {}
    def par(name, shape, dtype):
        P[name] = nc.dram_tensor(name, shape, dtype, kind="ExternalInput")
    par("AGG", [128, NT, 128], dt.float8e4)
    par("EXPT", [128, NT, 128], dt.float8e4)
    par("XSE", [128, NT, 4], dt.bfloat16)
    par("XDE", [128, NT, 3], dt.bfloat16)
    par("IDX", [128, NT * 8], dt.int16)
    par("SEL", [128, NBLK, 2], dt.bfloat16)
    par("V", [12, 256], dt.bfloat16)
    par("W2CB", [128, 2, C], dt.bfloat16)
    par("A2D2", [C, 2], dt.bfloat16)
    par("B1SB", [128, 2], dt.float32)
    par("B2ROW", [1, C], dt.float32)
    par("WPT", [2, C], dt.float32)
    par("WVT", [1, C], dt.float32)
    par("BP", [1, 2], dt.float32)
    par("BV", [1, 1], dt.float32)
    par("W1SB", [3, 256], dt.float32)
    par("A1F", [1, 256], dt.float32)
    par("D1F", [1, 256], dt.float32)
    par("IDEN", [128, 128], dt.bfloat16)
    par("IDENF", [4, 4], dt.float32)
    out_ext = nc.dram_tensor("out", [1, 3], dt.float32, kind="ExternalOutput")
    dbg1_ext = nc.dram_tensor("dbg1", [128, NBLK, 16], dt.float32, kind="ExternalOutput")
    dbg2_ext = nc.dram_tensor("dbg2", [128, NBLK, 66], dt.float32, kind="ExternalOutput")
    dbg3_ext = nc.dram_tensor("dbg3", [128, 64], dt.float32, kind="ExternalOutput")
    dbg4_ext = nc.dram_tensor("dbg4", [128, 128], dt.bfloat16, kind="ExternalOutput")
    dbg5_ext = nc.dram_tensor("dbg5", [128, 128], dt.bfloat16, kind="ExternalOutput")

    recbounce = nc.dram_tensor("recbounce", [DSH, 128], dt.bfloat16)
    table = nc.dram_tensor("tableag", [N, 128], dt.bfloat16, addr_space="Shared")
    finloc = nc.dram_tensor("finloc", [2, C], dt.float32)
    finsh = nc.dram_tensor("finsh", [2, C], dt.float32, addr_space="Shared")

    NCH = (NBLK * 128) // 512   # 5
    NBATCH = NCALL // CPB

    with nc.semaphore("cs") as cs, nc.semaphore("cc") as cc, \
         tile.TileContext(nc) as tc, ExitStack() as ctx:
        pool = ctx.enter_context(tc.tile_pool(name="persist", bufs=1))
        stream = ctx.enter_context(tc.tile_pool(name="stream", bufs=3))
        ring = ctx.enter_context(tc.tile_pool(name="ring", bufs=3))
        s2p = ctx.enter_context(tc.tile_pool(name="s2p", bufs=4))
        pps = ctx.enter_context(tc.tile_pool(name="pps", bufs=2, space="PSUM"))
        ppb = pps
        ppq = ctx.enter_context(tc.tile_pool(name="ppq", bufs=2, space="PSUM"))

        def T(name, shape, dtype):
            t = pool.tile(shape, dtype, tag=name)
            return t

        # persistent SBUF tiles + input loads
        loads = [
            ("XSE", [128, NT, 4], dt.bfloat16), ("XDE", [128, NT, 3], dt.bfloat16),
            ("IDX", [128, NT * 8], dt.int16), ("SEL", [128, NBLK, 2], dt.bfloat16),
            ("V", [12, 256], dt.bfloat16), ("W2CB", [128, 2, C], dt.bfloat16),
            ("A2D2", [C, 2], dt.bfloat16), ("B1SB", [128, 2], dt.float32),
            ("B2ROW", [1, C], dt.float32), ("WPT", [2, C], dt.float32),
            ("WVT", [1, C], dt.float32), ("BP", [1, 2], dt.float32),
            ("BV", [1, 1], dt.float32), ("W1SB", [3, 256], dt.float32),
            ("A1F", [1, 256], dt.float32), ("D1F", [1, 256], dt.float32),
            ("IDEN", [128, 128], dt.bfloat16), ("IDENF", [4, 4], dt.float32),
        ]
        S = {}
        for name, shape, dtype in loads:
            S[name] = T(name, shape, dtype)
            nc.sync.dma_start(out=S[name][:], in_=P[name][:])

        TT = nc.vector.tensor_tensor
        ACT = nc.scalar.activation

        # ---- v1/u1 = einsum(W1,[a_src1|a_dst1]) -> broadcast to all partitions
        tmp = pool.tile([3, 256], dt.float32, tag="tmp")
        v1s = pool.tile([3, 4], dt.float32, tag="v1s")
        u1s = pool.tile([3, 4], dt.float32, tag="u1s")
        vpack = pool.tile([1, 24], dt.float32, tag="vpack")
        ones1 = pool.tile([1, 128], dt.float32, tag="ones1")
        v1bc = pool.tile([128, 24], dt.float32, tag="v1bc")
        nc.vector.memset(ones1[:], 1.0)
        rep = pool.tile([3, 256], dt.float32, tag="rep")
        for f in range(3):
            nc.gpsimd.dma_start(out=rep[f:f + 1, :], in_=S["A1F"][:])
        TT(out=tmp[:], in0=S["W1SB"][:], in1=rep[:], op=AT.mult)
        nc.vector.reduce_sum(out=v1s[:].rearrange("f (h o) -> f h o", o=1),
                             in_=tmp[:].rearrange("f (h c) -> f h c", h=4), axis=AX.X)
        for f in range(3):
            nc.gpsimd.dma_start(out=rep[f:f + 1, :], in_=S["D1F"][:])
        TT(out=tmp[:], in0=S["W1SB"][:], in1=rep[:], op=AT.mult)
        nc.vector.reduce_sum(out=u1s[:].rearrange("f (h o) -> f h o", o=1),
                             in_=tmp[:].rearrange("f (h c) -> f h c", h=4), axis=AX.X)
        nc.gpsimd.dma_start(out=vpack[0:1, 0:12], in_=v1s[:])
        nc.gpsimd.dma_start(out=vpack[0:1, 12:24], in_=u1s[:])
        pv = pps.tile([128, 24], dt.float32, tag="ps")
        nc.tensor.matmul(out=pv[:], lhsT=ones1[:], rhs=vpack[:], start=True, stop=True)
        ACT(out=v1bc[:], in_=pv[:], func=AF.Copy)

        # ---- layer-1 per-edge attention scalars
        AS1 = T("AS1", [128, NT, 4], dt.bfloat16)
        AD1 = T("AD1", [128, NT, 4], dt.bfloat16)
        TMP4 = T("TMP4", [128, NT, 4], dt.bfloat16)
        S1 = T("S1", [128, NT, 4], dt.bfloat16)
        MSG1 = T("MSG1", [128, NT, 16], dt.bfloat16)
        for (dstt, srcs, base) in [(AS1, S["XSE"], 0), (AD1, S["XDE"], 12)]:
            for f in range(3):
                t_in0 = srcs[:, :, f:f + 1].broadcast_to([128, NT, 4])
                t_sc = v1bc[:, base + f * 4:base + f * 4 + 4] \
                    .rearrange("p (o h) -> p o h", o=1).broadcast_to([128, NT, 4])
                if f == 0:
                    TT(out=dstt[:], in0=t_in0, in1=t_sc, op=AT.mult)
                else:
                    TT(out=TMP4[:], in0=t_in0, in1=t_sc, op=AT.mult)
                    TT(out=dstt[:], in0=dstt[:], in1=TMP4[:], op=AT.add)
        TT(out=AS1[:], in0=AS1[:], in1=AD1[:], op=AT.add)      # e1
        nc.vector.tensor_scalar_mul(out=AD1[:], in0=AS1[:], scalar1=0.2)
        TT(out=AS1[:], in0=AS1[:], in1=AD1[:], op=AT.max)      # lrelu(e1)
        ACT(out=S1[:], in_=AS1[:], func=AF.Exp)
        TT(out=MSG1[:],
           in0=S1[:].rearrange("p t (h o) -> p t h o", o=1).broadcast_to([128, NT, 4, 4]),
           in1=S["XSE"][:].rearrange("p (t o) f -> p t o f", o=1).broadcast_to([128, NT, 4, 4]),
           op=AT.mult)

        # ---- layer-1 aggregation (AGG-streamed interval matmuls)
        Q1 = T("Q1", [128, NBLK, 16], dt.float32)
        blk_of = lambda t: min(t // TB, NBLK - 1)
        pq = None
        for bb in range(NT // 32):
            ag = stream.tile([128, 32, 128], dt.float8e4, tag="aggs")
            nc.sync.dma_start(out=ag[:], in_=P["AGG"][:, bb * 32:(bb + 1) * 32, :])
            for j in range(32):
                t = bb * 32 + j
                b = blk_of(t)
                first = (t == b * TB) or (b == NBLK - 1 and t == (NBLK - 1) * TB)
                last = (t == (b + 1) * TB - 1) or (t == NT - 1)
                if b == NBLK - 1:
                    last = (t == NT - 1)
                if first:
                    pq = ppq.tile([128, 16], dt.float32, tag="pq")
                nc.tensor.matmul(out=pq[:], lhsT=ag[:, j, :], rhs=MSG1[:, t, :],
                                 start=first, stop=last)
                if last:
                    ACT(out=Q1[:, b, :], in_=pq[:], func=AF.Copy)

        # ---- normalize, out1 = relu(yn @ V + b1), h2 = x1 @ W2, as2/ad2
        ZR = T("ZR", [128, NBLK, 4], dt.float32)
        YN = T("YN", [128, NBLK, 16], dt.bfloat16)
        nc.vector.tensor_scalar_add(out=ZR[:], in0=Q1[:, :, 3::4], scalar1=1e-16)
        nc.vector.reciprocal(out=ZR[:], in_=ZR[:])
        TT(out=YN[:], in0=Q1[:],
           in1=ZR[:].rearrange("p b (h o) -> p b h o", o=1).broadcast_to([128, NBLK, 4, 4]),
           op=AT.mult)
        YNT = T("YNT", [12, NBLK * 128], dt.bfloat16)
        YN12 = T("YN12", [128, NBLK, 12], dt.bfloat16)
        nc.vector.tensor_copy(out=YN12[:].rearrange("p b (h f) -> p b h f", h=4),
                              in_=YN[:].rearrange("p b (h f) -> p b h f", h=4)[:, :, :, 0:3])
        for b in range(NBLK):
            pt = pps.tile([12, 128], dt.bfloat16, tag="ps")
            nc.tensor.transpose(out=pt[:], in_=YN12[:, b, :], identity=S["IDEN"][:])
            ACT(out=YNT[:, b * 128:(b + 1) * 128], in_=pt[:], func=AF.Copy)
        X1T = T("X1T", [128, 2, NBLK * 128], dt.bfloat16)
        for j in range(2):
            for chv in range(NCH):
                pb = ppb.tile([128, 512], dt.float32, tag="ps")
                nc.tensor.matmul(out=pb[:], lhsT=S["V"][:, j * 128:(j + 1) * 128],
                                 rhs=YNT[:, chv * 512:(chv + 1) * 512], start=True, stop=True)
                ACT(out=X1T[:, j, chv * 512:(chv + 1) * 512], in_=pb[:], func=AF.Relu,
                    bias=S["B1SB"][:, j:j + 1], scale=1.0)
        H2T = T("H2T", [C, NBLK * 128], dt.bfloat16)
        for chv in range(NCH):
            pb = ppb.tile([64, 512], dt.float32, tag="ps")
            for j in range(2):
                nc.tensor.matmul(out=pb[:], lhsT=S["W2CB"][:, j, :],
                                 rhs=X1T[:, j, chv * 512:(chv + 1) * 512],
                                 start=(j == 0), stop=(j == 1))
            ACT(out=H2T[:, chv * 512:(chv + 1) * 512], in_=pb[:], func=AF.Copy)
        ASD = T("ASD", [2, NBLK * 128], dt.float32)
        for chv in range(NCH):
            pb = pps.tile([2, 512], dt.float32, tag="ps")
            nc.tensor.matmul(out=pb[:], lhsT=S["A2D2"][:], rhs=H2T[:, chv * 512:(chv + 1) * 512],
                             start=True, stop=True)
            ACT(out=ASD[:, chv * 512:(chv + 1) * 512], in_=pb[:], func=AF.Copy)

        # ---- records [h2 | 1 | as2 | 0pad] bf16 and table allgather
        REC = T("REC", [128, NBLK, 128], dt.bfloat16)
        nc.vector.memset(REC[:, :, 64:128], 0.0)
        nc.vector.memset(REC[:, :, 64:65], 1.0)
        for b in range(NBLK):
            pt = pps.tile([128, 64], dt.bfloat16, tag="ps")
            nc.tensor.transpose(out=pt[:], in_=H2T[:, b * 128:(b + 1) * 128], identity=S["IDEN"][0:64, 0:64])
            ACT(out=REC[:, b, 0:64], in_=pt[:], func=AF.Copy)
        AS2C = T("AS2C", [128, NBLK], dt.float32)
        AD2C = T("AD2C", [128, NBLK], dt.bfloat16)
        ASDC = T("ASDC", [128, NBLK, 2], dt.float32)
        for b in range(NBLK):
            pt2 = pps.tile([128, 2], dt.float32, tag="ps")
            nc.tensor.transpose(out=pt2[:], in_=ASD[:, b * 128:(b + 1) * 128], identity=S["IDENF"][0:2, 0:2])
            ACT(out=ASDC[:, b, :], in_=pt2[:], func=AF.Copy)
        nc.vector.tensor_copy(out=AS2C[:], in_=ASDC[:, :, 0])
        nc.vector.tensor_copy(out=AD2C[:], in_=ASDC[:, :, 1])
        nc.vector.tensor_copy(out=REC[:, :, 65:66], in_=AS2C[:].rearrange("p (b o) -> p b o", o=1))

        # ---- ad2 per-edge expansion via transposed interval matmuls
        AD2E = T("AD2E", [128, NT], dt.float32)
        for bb in range(NT // 32):
            ex = stream.tile([128, 32, 128], dt.float8e4, tag="aggs")
            nc.sync.dma_start(out=ex[:], in_=P["EXPT"][:, bb * 32:(bb + 1) * 32, :])
            pb = ppb.tile([128, 32], dt.float32, tag="ps")
            for j in range(32):
                t = bb * 32 + j
                b = blk_of(t)
                nc.tensor.matmul(out=pb[:, j:j + 1], lhsT=ex[:, j, :],
                                 rhs=AD2C[:, b:b + 1], start=True, stop=True)
            ACT(out=AD2E[:, bb * 32:(bb + 1) * 32], in_=pb[:], func=AF.Copy)

        with tc.tile_critical():
            nc.gpsimd.dma_start(out=recbounce[0:19 * 128, :].rearrange("(b p) c -> p b c", p=128),
                                in_=REC[:, 0:19, :]).then_inc(cs, 16)
            nc.gpsimd.dma_start(out=recbounce[19 * 128:DSH, :],
                                in_=REC[0:DSH - 19 * 128, 19, :]).then_inc(cs, 16)
            nc.gpsimd.wait_ge(cs, 32)
            nc.gpsimd.collective_compute(
                "AllGather", AT.bypass, replica_groups=[list(range(NCORE))],
                ins=[recbounce[:]], outs=[table[:]]).then_inc(cc, 1)
            nc.gpsimd.wait_ge(cc, 1)

        # ---- layer-2: gather + softmax scale + aggregation
        Q2 = T("Q2", [128, NBLK, 66], dt.float32)
        pq2 = None
        for bt in range(NBATCH):
            gr = ring.tile([128, CPB * 8, 128], dt.bfloat16, tag="gr")
            for cll in range(CPB):
                g = bt * CPB + cll
                nc.gpsimd.dma_gather(
                    gr[:, cll * 8:(cll + 1) * 8, :], table[:, :],
                    S["IDX"][:, g * 64:(g + 1) * 64],
                    GCALL, GCALL, 128, queue_num=g % 4)
            s2 = s2p.tile([128, CPB * 8], dt.float32, tag="s2")
            s2b = s2p.tile([128, CPB * 8], dt.float32, tag="s2")
            a2c = gr[:, :, 65:66].rearrange("p t o -> p (t o)")
            e2c = AD2E[:, bt * 32:(bt + 1) * 32]
            TT(out=s2[:], in0=a2c, in1=e2c, op=AT.add)
            nc.vector.tensor_scalar_mul(out=s2b[:], in0=s2[:], scalar1=0.2)
            TT(out=s2[:], in0=s2[:], in1=s2b[:], op=AT.max)
            ACT(out=s2[:], in_=s2[:], func=AF.Exp)
            TT(out=gr[:, :, 0:66], in0=gr[:, :, 0:66],
               in1=s2[:].rearrange("p (t o) -> p t o", o=1).broadcast_to([128, CPB * 8, 66]),
               op=AT.mult)
            ag = stream.tile([128, 32, 128], dt.float8e4, tag="aggs")
            nc.sync.dma_start(out=ag[:], in_=P["AGG"][:, bt * 32:(bt + 1) * 32, :])
            for j in range(32):
                t = bt * 32 + j
                b = blk_of(t)
                first = (t == b * TB) or (b == NBLK - 1 and t == (NBLK - 1) * TB)
                last = (t == (b + 1) * TB - 1) or (t == NT - 1)
                if b == NBLK - 1:
                    last = (t == NT - 1)
                if first:
                    pq2 = ppq.tile([128, 66], dt.float32, tag="pq")
                nc.tensor.matmul(out=pq2[:], lhsT=ag[:, j, :], rhs=gr[:, j, 0:66],
                                 start=first, stop=last)
                if last:
                    ACT(out=Q2[:, b, :], in_=pq2[:], func=AF.Copy)

        # ---- emb, heads partials, allreduce, final
        B2BC = T("B2BC", [128, C], dt.float32)
        pb2 = pps.tile([128, C], dt.float32, tag="ps")
        nc.tensor.matmul(out=pb2[:], lhsT=ones1[:], rhs=S["B2ROW"][:], start=True, stop=True)
        ACT(out=B2BC[:], in_=pb2[:], func=AF.Copy)
        EMB = T("EMB", [128, NBLK, C], dt.bfloat16)
        ZR2 = T("ZR2", [128, NBLK, 1], dt.float32)
        nc.vector.tensor_scalar_add(out=ZR2[:], in0=Q2[:, :, 64:65], scalar1=1e-16)
        nc.vector.reciprocal(out=ZR2[:], in_=ZR2[:])
        TT(out=EMB[:], in0=Q2[:, :, 0:64], in1=ZR2[:].broadcast_to([128, NBLK, 64]), op=AT.mult)
        TT(out=EMB[:], in0=EMB[:],
           in1=B2BC[:].rearrange("p (z c) -> p z c", z=1).broadcast_to([128, NBLK, 64]),
           op=AT.add)
        pf = pps.tile([2, C], dt.float32, tag="ps")
        for b in range(NBLK):
            nc.tensor.matmul(out=pf[:], lhsT=S["SEL"][:, b, :], rhs=EMB[:, b, :],
                             start=(b == 0), stop=(b == NBLK - 1))
        FIN = T("FIN", [2, C], dt.float32)
        SC1 = T("SC1", [2, C], dt.float32)
        ACT(out=FIN[:], in_=pf[:], func=AF.Copy)
        with tc.tile_critical():
            nc.gpsimd.dma_start(out=finloc[:], in_=FIN[:]).then_inc(cs, 16)
            nc.gpsimd.wait_ge(cs, 48)
            nc.gpsimd.collective_compute(
                "AllReduce", AT.add, replica_groups=[list(range(NCORE))],
                ins=[finloc[:]], outs=[finsh[:]]).then_inc(cc, 1)
            nc.gpsimd.wait_ge(cc, 2)
            nc.gpsimd.dma_start(out=SC1[:], in_=finsh[:]).then_inc(cs, 16)
            nc.gpsimd.wait_ge(cs, 64)

        WPR = T("WPR", [1, 2 * C], dt.float32)
        nc.gpsimd.dma_start(out=WPR[:], in_=S["WPT"][:])
        SC1F = T("SC1F", [1, 2 * C], dt.float32)
        nc.gpsimd.dma_start(out=SC1F[:], in_=SC1[:])
        TMP2 = T("TMP2", [2, C], dt.float32)
        SC2 = T("SC2", [2, 1], dt.float32)
        SC3 = T("SC3", [1, 4], dt.float32)
        OUTS = T("OUTS", [1, 3], dt.float32)
        for j in range(2):
            TT(out=TMP2[0:1, :], in0=WPR[0:1, j * C:(j + 1) * C], in1=SC1F[0:1, 0:C], op=AT.mult)
            nc.vector.reduce_sum(out=SC3[0:1, j:j + 1], in_=TMP2[0:1, :], axis=AX.X)
        TT(out=SC3[0:1, 0:2], in0=SC3[0:1, 0:2], in1=S["BP"][:], op=AT.add)
        TT(out=TMP2[0:1, :], in0=S["WVT"][:], in1=SC1F[0:1, C:2 * C], op=AT.mult)
        nc.vector.reduce_sum(out=SC3[0:1, 2:3], in_=TMP2[0:1, :], axis=AX.X)
        nc.vector.tensor_scalar_mul(out=SC3[0:1, 2:3], in0=SC3[0:1, 2:3], scalar1=1.0 / N)
        TT(out=SC3[0:1, 2:3], in0=SC3[0:1, 2:3], in1=S["BV"][:], op=AT.add)
        ACT(out=SC3[0:1, 0:2], in_=SC3[0:1, 0:2], func=AF.Exp)
        nc.vector.reduce_sum(out=SC3[0:1, 3:4], in_=SC3[0:1, 0:2], axis=AX.X)
        nc.vector.reciprocal(out=SC3[0:1, 3:4], in_=SC3[0:1, 3:4])
        TT(out=OUTS[0:1, 0:2], in0=SC3[0:1, 0:2],
           in1=SC3[0:1, 3:4].broadcast_to([1, 2]), op=AT.mult)
        nc.vector.tensor_copy(out=OUTS[0:1, 2:3], in_=SC3[0:1, 2:3])
        nc.gpsimd.dma_start(out=out_ext[:], in_=OUTS[:])
        nc.gpsimd.dma_start(out=dbg1_ext[:], in_=Q1[:])
        nc.gpsimd.dma_start(out=dbg2_ext[:], in_=Q2[:])
        nc.gpsimd.dma_start(out=dbg3_ext[:, 0:NBLK], in_=AD2E[:, 0:NBLK])
        nc.gpsimd.dma_start(out=dbg4_ext[:], in_=REC[:, 0, :])
        nc.gpsimd.dma_start(out=dbg5_ext[:], in_=table[128:256, :])

    nc.compile()
    return nc


def kernel(**inputs):
    from concourse.bass_utils import run_bass_kernel_spmd
    NT, NCALL, TB, in_maps = build_plan(inputs)
    key = (NT, NCALL, TB)
    if key not in _graph_cache:
        _graph_cache[key] = build_graph(NT, NCALL, TB)
    nc = _graph_cache[key]
    res = run_bass_kernel_spmd(nc, in_maps, list(range(NCORE)))
    o = np.asarray(res.results[0]["out"], dtype=np.float32).reshape(3)
    return np.asarray(o[0:2], dtype=np.float32), np.asarray(o[2:3], dtype=np.float32)
